# revision 45
# baseline (speedup 1.0000x reference)
"""Trainium2 Bass kernel for nn_EnhancedMathematicalReasoning.

Full (unsharded) inputs in, full outputs out. Internally: data-parallel over
batch (B=8) across 8 NeuronCores; each core runs the whole transformer block +
top-1 domain routing for its batch element. Routing uses an on-device argmax
and a dynamic-offset DMA so only the selected domain head's GEMM is computed.

Layout conventions (per core):
  - "T-layout" activations: [feature_part(128), feature_tile(kt), seq] in SBUF.
  - GEMM operands bf16, accumulation fp32 in PSUM.
  - out = lhsT.T @ rhs with lhsT [K<=128, M<=128], rhs [K<=128, N].
"""

import os
import sys

sys.path.insert(0, "/opt/trn_rl_repo")

import numpy as np
import ml_dtypes

import concourse.bass as bass
import concourse.mybir as mybir
import concourse.tile as tile
from concourse import bacc
from concourse.bass import ds

F32 = mybir.dt.float32
F32R = mybir.dt.float32r
BF16 = mybir.dt.bfloat16
I32 = mybir.dt.int32
AF = mybir.ActivationFunctionType
ALU = mybir.AluOpType
AX = mybir.AxisListType

B, S, H, NH, DH, DFF, ND = 8, 2048, 1024, 8, 128, 4096, 5
KT = H // 128          # 8 feature tiles
ST = S // 128          # 16 seq tiles
DT = DFF // 128        # 32 ffn tiles
NC = 512               # gemm seq chunk
NCH = S // NC          # 4
FC = 256               # ffn seq chunk
FCH = S // FC          # 8
INV_SQRT_DH = 1.0 / float(np.sqrt(DH))
EPS = 1e-5

_CACHE = {}


def _t8(w):
    """[1024, X...] -> [128, 8, X...] tiled along rows."""
    w = np.asarray(w)
    return np.ascontiguousarray(
        w.reshape(8, 128, *w.shape[1:]).transpose(1, 0, *range(2, w.ndim + 1)))


def _bvec(v, t):
    v = np.asarray(v, np.float32)
    return np.ascontiguousarray(v.reshape(t, 128).T)


def _bf(w):
    return np.asarray(w).astype(ml_dtypes.bfloat16)


def build_nc(dbg=False):
    nc = bacc.Bacc("TRN2", target_bir_lowering=False, debug=False, num_devices=8)

    di = {}

    def inp(name, shape, dtype):
        di[name] = nc.dram_tensor(name, list(shape), dtype, kind="ExternalInput")
        return di[name]

    # inputs (per core)
    inp("hT", [128, KT, S], BF16)
    for w in ["w_in", "wq", "wk", "wv", "wo", "w_outp"]:
        inp(w, [128, KT, H], BF16)
    inp("w1", [128, DT, KT, 128], BF16)
    inp("w2", [128, DT, H], BF16)
    inp("w_heads", [ND, 128, KT, H], BF16)
    inp("wu1", [128, KT, 512], BF16)
    inp("wu2", [128, 4], BF16)
    inp("w_dom", [128, KT, ND], F32)
    inp("cpack", [128, 10 * KT + DT + 4], F32)
    inp("rpack", [1, 16], F32)
    inp("bh", [ND, 128, KT], F32)
    inp("b_outp_row", [1, H], BF16)

    out_p = nc.dram_tensor("out_p", [S, H], F32, kind="ExternalOutput")
    out_probs = nc.dram_tensor("out_probs", [1, ND], F32, kind="ExternalOutput")
    out_idx = nc.dram_tensor("out_idx", [1, 1], I32, kind="ExternalOutput")
    out_unc = nc.dram_tensor("out_unc", [1, 1], F32, kind="ExternalOutput")

    dbg_outs = {}
    if dbg:
        for nm in ["xT", "aoT", "r1", "ln1", "r2", "yT", "cT"]:
            dbg_outs[nm] = nc.dram_tensor("dbg_" + nm, [128, KT, S], BF16,
                                          kind="ExternalOutput")
        dbg_outs["whead"] = nc.dram_tensor("dbg_whead", [128, KT, H], BF16,
                                           kind="ExternalOutput")
        dbg_outs["woutp"] = nc.dram_tensor("dbg_woutp", [128, KT, H], BF16,
                                           kind="ExternalOutput")
        dbg_outs["bob"] = nc.dram_tensor("dbg_bob", [128, H], F32,
                                         kind="ExternalOutput")
    with tile.TileContext(nc) as tc:
        _emit(nc, tc, di, out_p, out_probs, out_idx, out_unc, dbg_outs)
    nc.compile()
    return nc


def _emit(nc, tc, di, out_p, out_probs, out_idx, out_unc, dbg_outs={}):
    dma = nc.sync.dma_start
    wdma = nc.scalar.dma_start

    def dbg_dump(nm, t):
        if nm in dbg_outs:
            dma(dbg_outs[nm].ap()[:], t[:])

    # ---- pools with whole-kernel lifetime (opened first, closed last) ----
    cpool_cm = tc.tile_pool(name="cpool", bufs=1)
    cp = cpool_cm.__enter__()
    # 4MB activation slots, rotating: hT, xT, aoT, r1, ln1, ffA, r2, yT, cT
    big_cm = tc.tile_pool(name="big", bufs=3)
    bigp = big_cm.__enter__()
    # 2MB weight slots, rotating: w_in, wv, wq, wk, wo, whead, w_outp
    wts_cm = tc.tile_pool(name="wts", bufs=3)
    wts = wts_cm.__enter__()

    cnt = {"big": 0, "w": 0}

    def big_tile():
        cnt["big"] += 1
        return bigp.tile([128, KT, S], BF16, tag="act", name=f"act{cnt['big']}")

    def w_tile(name, dyn_rv=None, split_m=False):
        cnt["w"] += 1
        t = wts.tile([128, KT, H], BF16, tag="w", name=f"w{cnt['w']}")
        if dyn_rv is None:
            if split_m:
                for mm in range(KT):
                    msl = slice(mm * 128, (mm + 1) * 128)
                    dma(t[:, :, msl], di[name].ap()[:, :, msl])
            else:
                dma(t[:], di[name].ap()[:])
        else:
            nc.gpsimd.dma_start(
                t[:],
                di[name].ap()[ds(dyn_rv, 1), :, :, :].rearrange(
                    "a p k o -> p (a k) o"),
            )
        return t

    def load_const(name):
        t = cp.tile(list(di[name].shape), di[name].dtype, tag=name)
        dma(t[:], di[name].ap()[:])
        return t

    # critical-path DMAs first (same HWDGE queue is FIFO)
    hT = big_tile()
    dma(hT[:, :, 0:NC], di["hT"].ap()[:, :, 0:NC])
    w_in = w_tile("w_in", split_m=True)
    for n in range(1, NCH):
        nsl = slice(n * NC, (n + 1) * NC)
        dma(hT[:, :, nsl], di["hT"].ap()[:, :, nsl])

    cpk = load_const("cpack")
    names10 = ["b_in", "bq", "bk", "bo", "b2s", "b_outp", "ln1g", "ln1b",
               "ln2g", "ln2b"]
    cs = {}
    for i, nm in enumerate(names10):
        cs[nm] = cpk[:, i * KT:(i + 1) * KT]
    b_in = cs["b_in"]; bq = cs["bq"]; bk = cs["bk"]; bo = cs["bo"]
    b2s = cs["b2s"]; b_outp = cs["b_outp"]
    ln1g = cs["ln1g"]; ln1b = cs["ln1b"]; ln2g = cs["ln2g"]; ln2b = cs["ln2b"]
    b1s = cpk[:, 10 * KT:10 * KT + DT]
    bu1 = cpk[:, 10 * KT + DT:10 * KT + DT + 4]
    rpk = load_const("rpack")
    b_dom = rpk[:, 0:ND]
    iota5 = rpk[:, ND:2 * ND]
    bu2 = rpk[:, 2 * ND:2 * ND + 1]
    bh = load_const("bh")
    b_outp_row = load_const("b_outp_row")

    ones_bf = cp.tile([128, 1], BF16, tag="ones_bf")
    nc.vector.memset(ones_bf[:], 1.0)
    ones_bfrow = cp.tile([1, 128], BF16, tag="ones_bfrow")
    nc.vector.memset(ones_bfrow[:], 1.0)
    ones_row32 = cp.tile([1, 128], F32, tag="ones_row32")
    nc.vector.memset(ones_row32[:], 1.0)
    ones_row = cp.tile([1, 128], F32R, tag="ones_row")
    with nc.allow_low_precision(reason="f32r ones for broadcast matmul"):
        nc.vector.tensor_copy(ones_row[:], ones_row32[:])
    eps_t = cp.tile([1, 1], F32, tag="eps")
    nc.vector.memset(eps_t[:], EPS)


    # ================= stage 1: x^T = hidden @ W_in + b_in =================
    # (tiles created above; GEMM loop below after layer_norm def)

    # ================= layernorm helper =================
    def layer_norm(src, dst, g_sb, bb_sb, tagp, chunk_cb=None, LNC=512,
                   chunks=None):
        ln_cm = tc.tile_pool(name="ln" + tagp, bufs=2)
        lnp = ln_cm.__enter__()
        st_cm = tc.tile_pool(name="lnst" + tagp, bufs=2, space="PSUM")
        stp = st_cm.__enter__()
        bc_cm = tc.tile_pool(name="lnbc" + tagp, bufs=2, space="PSUM")
        bcp = bc_cm.__enter__()
        if chunks is None:
            chunks = [(i * LNC, LNC) for i in range(S // LNC)]
        for n, (c0, LNC) in enumerate(chunks):
            nsl = slice(c0, c0 + LNC)
            sq = lnp.tile([128, KT, LNC], BF16, tag="sq")
            for k in range(KT):
                nc.scalar.activation(sq[:, k, :], src[:, k, nsl], AF.Square)
            sum_ps = stp.tile([1, LNC], F32, tag="st")
            sq_ps = stp.tile([1, LNC], F32, tag="st")
            for k in range(KT):
                nc.tensor.matmul(sum_ps[:], ones_bf[:], src[:, k, nsl],
                                 start=(k == 0), stop=(k == KT - 1))
                nc.tensor.matmul(sq_ps[:], ones_bf[:], sq[:, k, :],
                                 start=(k == 0), stop=(k == KT - 1))
            m_sb = lnp.tile([1, LNC], F32R, tag="m")
            nc.vector.tensor_scalar_mul(m_sb[:], sum_ps[:], 1.0 / H)
            var = lnp.tile([1, LNC], F32, tag="var")
            nc.vector.tensor_scalar_mul(var[:], sq_ps[:], 1.0 / H)
            msq = lnp.tile([1, LNC], F32, tag="msq")
            nc.vector.tensor_tensor(msq[:], m_sb[:], m_sb[:], op=ALU.mult)
            nc.vector.tensor_tensor(var[:], var[:], msq[:], op=ALU.subtract)
            sd = lnp.tile([1, LNC], F32, tag="sd")
            nc.scalar.activation(sd[:], var[:], AF.Sqrt, bias=eps_t[:])
            rs = lnp.tile([1, LNC], F32R, tag="rs")
            with nc.allow_low_precision(reason="f32r for broadcast matmul"):
                nc.vector.reciprocal(rs[:], sd[:])
            mb_ps = bcp.tile([128, LNC], F32, tag="bc")
            nc.tensor.matmul(mb_ps[:], ones_row[:], m_sb[:], start=True, stop=True)
            mb = lnp.tile([128, LNC], F32, tag="mb")
            nc.scalar.copy(mb[:], mb_ps[:])
            rb_ps = bcp.tile([128, LNC], F32, tag="bc")
            nc.tensor.matmul(rb_ps[:], ones_row[:], rs[:], start=True, stop=True)
            rb = lnp.tile([128, LNC], F32, tag="rb")
            nc.scalar.copy(rb[:], rb_ps[:])
            for k in range(KT):
                t1 = lnp.tile([128, LNC], F32, tag="t1")
                nc.vector.tensor_tensor(t1[:], src[:, k, nsl], mb[:], op=ALU.subtract)
                nc.vector.tensor_tensor(t1[:], t1[:], rb[:], op=ALU.mult)
                nc.scalar.activation(dst[:, k, nsl], t1[:], AF.Identity,
                                     bias=bb_sb[:, k:k + 1], scale=g_sb[:, k:k + 1])
            if chunk_cb is not None:
                chunk_cb(n, nsl)
        bc_cm.__exit__(None, None, None)
        st_cm.__exit__(None, None, None)
        ln_cm.__exit__(None, None, None)

    xT = big_tile()

    gps_cm = tc.tile_pool(name="gps", bufs=3, space="PSUM")
    gps = gps_cm.__enter__()

    for m in range(KT):
        for n in range(NCH):
            ps = gps.tile([128, NC], F32, tag="g")
            for k in range(KT):
                nc.tensor.matmul(ps[:], w_in[:, k, m * 128:(m + 1) * 128],
                                 hT[:, k, n * NC:(n + 1) * NC],
                                 start=(k == 0), stop=(k == KT - 1))
            nc.scalar.activation(xT[:, m, n * NC:(n + 1) * NC], ps[:],
                                 AF.Identity, bias=b_in[:, m:m + 1])

    dbg_dump("xT", xT)

    # ================= stage 3: attention (v computed per head) =============
    wv = w_tile("wv")
    wq = w_tile("wq")
    wk = w_tile("wk")
    aoT = big_tile()

    vh_cm = tc.tile_pool(name="vh", bufs=2)
    vhp = vh_cm.__enter__()
    qk_cm = tc.tile_pool(name="qk", bufs=2)
    qkp = qk_cm.__enter__()
    p_cm = tc.tile_pool(name="pp", bufs=6)
    pp = p_cm.__enter__()
    aps_cm = tc.tile_pool(name="aps", bufs=2, space="PSUM")
    aps = aps_cm.__enter__()
    lps_cm = tc.tile_pool(name="lps", bufs=2, space="PSUM")
    lps = lps_cm.__enter__()
    bcps_cm = tc.tile_pool(name="bcps", bufs=1, space="PSUM")
    bcps = bcps_cm.__enter__()

    for h in range(NH):
        hsl = slice(h * 128, (h + 1) * 128)
        vh = vhp.tile([128, ST, 128], BF16, tag="v")
        for st in range(ST):
            ps = gps.tile([128, NC], F32, tag="g")
            for k in range(KT):
                nc.tensor.matmul(ps[:, 0:128], xT[:, k, st * 128:(st + 1) * 128],
                                 wv[:, k, hsl], start=(k == 0), stop=(k == KT - 1))
            nc.scalar.copy(vh[:, st, :], ps[:, 0:128])
        qT = qkp.tile([128, S], BF16, tag="q")
        kTt = qkp.tile([128, S], BF16, tag="k")
        for dst, wmat, bias in ((qT, wq, bq), (kTt, wk, bk)):
            for n in range(NCH):
                ps = gps.tile([128, NC], F32, tag="g")
                for k in range(KT):
                    nc.tensor.matmul(ps[:], wmat[:, k, hsl],
                                     xT[:, k, n * NC:(n + 1) * NC],
                                     start=(k == 0), stop=(k == KT - 1))
                nc.scalar.activation(dst[:, n * NC:(n + 1) * NC], ps[:],
                                     AF.Identity, bias=bias[:, h:h + 1])
        for c in range(NCH):
            csl = slice(c * NC, (c + 1) * NC)
            ao_ps = aps.tile([128, NC], F32, tag="ao")
            l_ps = lps.tile([1, NC], F32, tag="l")
            for t in range(ST):
                sc_ps = gps.tile([128, NC], F32, tag="g")
                nc.tensor.matmul(sc_ps[:], kTt[:, t * 128:(t + 1) * 128],
                                 qT[:, csl], start=True, stop=True)
                p_sb = pp.tile([128, NC], BF16, tag="p")
                nc.scalar.activation(p_sb[:], sc_ps[:], AF.Exp, scale=INV_SQRT_DH)
                nc.tensor.matmul(ao_ps[:], vh[:, t, :], p_sb[:],
                                 start=(t == 0), stop=(t == ST - 1))
                nc.tensor.matmul(l_ps[:], ones_bf[:], p_sb[:],
                                 start=(t == 0), stop=(t == ST - 1))
            rl = pp.tile([1, NC], F32R, tag="rl")
            with nc.allow_low_precision(reason="f32r for broadcast matmul"):
                nc.vector.reciprocal(rl[:], l_ps[:])
            bc_ps = bcps.tile([128, NC], F32, tag="bc")
            nc.tensor.matmul(bc_ps[:], ones_row[:], rl[:], start=True, stop=True)
            bc_sb = pp.tile([128, NC], F32, tag="bcs")
            nc.scalar.copy(bc_sb[:], bc_ps[:])
            nc.vector.tensor_tensor(aoT[:, h, csl], ao_ps[:], bc_sb[:], op=ALU.mult)

    bcps_cm.__exit__(None, None, None)
    lps_cm.__exit__(None, None, None)
    aps_cm.__exit__(None, None, None)
    p_cm.__exit__(None, None, None)
    qk_cm.__exit__(None, None, None)
    vh_cm.__exit__(None, None, None)

    dbg_dump("aoT", aoT)

    # ================= stage 4: attn out proj + residual =================
    wo = w_tile("wo")
    r1 = big_tile()

    at_cm = tc.tile_pool(name="at", bufs=4)
    atp = at_cm.__enter__()
    for m in range(KT):
        for n in range(NCH):
            nsl = slice(n * NC, (n + 1) * NC)
            ps = gps.tile([128, NC], F32, tag="g")
            for k in range(KT):
                nc.tensor.matmul(ps[:], wo[:, k, m * 128:(m + 1) * 128],
                                 aoT[:, k, nsl], start=(k == 0), stop=(k == KT - 1))
            at_sb = atp.tile([128, NC], BF16, tag="at")
            nc.scalar.activation(at_sb[:], ps[:], AF.Identity, bias=bo[:, m:m + 1])
            nc.vector.tensor_tensor(r1[:, m, nsl], at_sb[:], xT[:, m, nsl], op=ALU.add)
    at_cm.__exit__(None, None, None)

    # ================= stage 5: LN1 =================
    dbg_dump("r1", r1)
    ln1 = big_tile()
    layer_norm(r1, ln1, ln1g, ln1b, "a")
    dbg_dump("ln1", ln1)

    gps_cm.__exit__(None, None, None)

    # ================= stage 6: FFN =================
    ffA = big_tile()
    r2 = big_tile()

    ffw_cm = tc.tile_pool(name="ffw", bufs=1)
    ffw = ffw_cm.__enter__()
    ffps_cm = tc.tile_pool(name="ffps", bufs=3, space="PSUM")
    ffps = ffps_cm.__enter__()
    h1ps_cm = tc.tile_pool(name="h1ps", bufs=3, space="PSUM")
    h1ps = h1ps_cm.__enter__()
    g_cm = tc.tile_pool(name="gel", bufs=3)
    gp = g_cm.__enter__()

    QDT = DT // 4  # 8 dff tiles per quarter
    for q in range(4):
        w1q = ffw.tile([128, QDT, KT, 128], BF16, tag="w1q", name=f"w1q{q}")
        w2q = ffw.tile([128, QDT, H], BF16, tag="w2q", name=f"w2q{q}")
        for dt in range(QDT):
            dma(w1q[:, dt, :, :], di["w1"].ap()[:, q * QDT + dt, :, :])
            dma(w2q[:, dt, :], di["w2"].ap()[:, q * QDT + dt, :])
        for c in range(FCH):
            csl = slice(c * FC, (c + 1) * FC)
            g_all = gp.tile([128, QDT, FC], BF16, tag="g", name=f"g_{q}_{c}")
            for dt in range(QDT):
                h1 = h1ps.tile([128, FC], F32, tag="h1")
                for k in range(KT):
                    nc.tensor.matmul(h1[:], w1q[:, dt, k, :], ln1[:, k, csl],
                                     start=(k == 0), stop=(k == KT - 1))
                nc.scalar.activation(g_all[:, dt, :], h1[:], AF.Gelu_apprx_tanh,
                                     bias=b1s[:, q * QDT + dt:q * QDT + dt + 1])
            for m in range(KT):
                ff_ps = ffps.tile([128, FC], F32, tag="ff")
                for dt in range(QDT):
                    nc.tensor.matmul(ff_ps[:], w2q[:, dt, m * 128:(m + 1) * 128],
                                     g_all[:, dt, :], start=(dt == 0),
                                     stop=(dt == QDT - 1))
                if q == 0:
                    nc.scalar.copy(ffA[:, m, csl], ff_ps[:])
                elif q < 3:
                    nc.vector.tensor_tensor(ffA[:, m, csl], ff_ps[:],
                                            ffA[:, m, csl], op=ALU.add)
                else:
                    fb = gp.tile([128, FC], BF16, tag="fb")
                    nc.scalar.activation(fb[:], ff_ps[:], AF.Identity,
                                         bias=b2s[:, m:m + 1])
                    t2 = gp.tile([128, FC], BF16, tag="t2")
                    nc.vector.tensor_tensor(t2[:], fb[:], ffA[:, m, csl], op=ALU.add)
                    nc.vector.tensor_tensor(r2[:, m, csl], t2[:], ln1[:, m, csl],
                                            op=ALU.add)

    g_cm.__exit__(None, None, None)
    h1ps_cm.__exit__(None, None, None)
    ffps_cm.__exit__(None, None, None)
    ffw_cm.__exit__(None, None, None)

    # ================= stage 7: LN2 -> y =================
    dbg_dump("r2", r2)
    yT = big_tile()
    ymp_cm = tc.tile_pool(name="ymp", bufs=1)
    ymp_p = ymp_cm.__enter__()
    ym_parts = ymp_p.tile([128, KT, 5], F32, tag="ymp")

    def _ym_cb(n, nsl):
        for k in range(KT):
            nc.vector.reduce_sum(ym_parts[:, k, n:n + 1], yT[:, k, nsl], axis=AX.X)

    layer_norm(r2, yT, ln2g, ln2b, "b", chunk_cb=_ym_cb)
    dbg_dump("yT", yT)

    # ================= stage 8+: tail (routing, head, outproj, uncertainty) ==
    tl_cm = tc.tile_pool(name="tail", bufs=1)
    tl = tl_cm.__enter__()
    ob_cm = tc.tile_pool(name="ob", bufs=4)
    obp = ob_cm.__enter__()
    ps2_cm = tc.tile_pool(name="ps2", bufs=3, space="PSUM")
    ps2 = ps2_cm.__enter__()
    sps_cm = tc.tile_pool(name="sps", bufs=1, space="PSUM")
    sps = sps_cm.__enter__()

    # ---- domain routing ----
    wdom = tl.tile([128, KT, ND], F32, tag="wdom")
    dma(wdom[:], di["w_dom"].ap()[:])
    ym = tl.tile([128, KT], F32, tag="ym")
    nc.vector.tensor_reduce(ym[:], ym_parts[:], axis=AX.X, op=ALU.add)
    nc.vector.tensor_scalar_mul(ym[:], ym[:], 1.0 / S)
    dl_ps = sps.tile([1, ND], F32, tag="dl")
    for k in range(KT):
        nc.tensor.matmul(dl_ps[:], ym[:, k:k + 1], wdom[:, k, :],
                         start=(k == 0), stop=(k == KT - 1))
    dl = tl.tile([1, ND], F32, tag="dls")
    nc.vector.tensor_tensor(dl[:], dl_ps[:], b_dom[:], op=ALU.add)
    mx = tl.tile([1, 1], F32, tag="mx")
    nc.vector.reduce_max(mx[:], dl[:], axis=AX.X)
    nmx = tl.tile([1, 1], F32, tag="nmx")
    nc.vector.tensor_scalar_mul(nmx[:], mx[:], -1.0)
    e5 = tl.tile([1, ND], F32, tag="e5")
    se = tl.tile([1, 1], F32, tag="se")
    nc.scalar.activation(e5[:], dl[:], AF.Exp, bias=nmx[:], accum_out=se[:])
    rse = tl.tile([1, 1], F32, tag="rse")
    nc.vector.reciprocal(rse[:], se[:])
    probs = tl.tile([1, ND], F32, tag="probs")
    nc.vector.tensor_scalar_mul(probs[:], e5[:], rse[:])
    dma(out_probs.ap()[:], probs[:])

    mask = tl.tile([1, ND], F32, tag="mask")
    nc.vector.tensor_scalar(mask[:], dl[:], mx[:], None, op0=ALU.is_ge)
    nc.vector.tensor_tensor(mask[:], mask[:], iota5[:], op=ALU.mult)
    idxf = tl.tile([1, 1], F32, tag="idxf")
    nc.vector.reduce_max(idxf[:], mask[:], axis=AX.X)
    idx_i = tl.tile([1, 1], I32, tag="idxi")
    nc.vector.tensor_copy(idx_i[:], idxf[:])
    dma(out_idx.ap()[:], idx_i[:])

    idx_rv = nc.values_load(
        idx_i[0:1, 0:1],
        engines=[mybir.EngineType.Pool, mybir.EngineType.Activation],
        min_val=0, max_val=ND - 1, skip_runtime_bounds_check=True,
    )

    # ---- selected domain head: combined^T = y @ W_heads[idx] + b_heads[idx]
    whead = w_tile("w_heads", dyn_rv=idx_rv)
    bh_sel = tl.tile([128, KT], F32, tag="bh_sel")
    nc.gpsimd.dma_start(
        bh_sel[:],
        di["bh"].ap()[ds(idx_rv, 1), :, :].rearrange("a p k -> p (a k)"))
    if "whead" in dbg_outs:
        dma(dbg_outs["whead"].ap()[:], whead[:])
    cT = big_tile()
    for m in range(KT):
        for n in range(NCH):
            nsl = slice(n * NC, (n + 1) * NC)
            ps = ps2.tile([128, NC], F32, tag="g")
            for k in range(KT):
                nc.tensor.matmul(ps[:], whead[:, k, m * 128:(m + 1) * 128],
                                 yT[:, k, nsl], start=(k == 0), stop=(k == KT - 1))
            nc.scalar.activation(cT[:, m, nsl], ps[:], AF.Identity,
                                 bias=bh_sel[:, m:m + 1])

    dbg_dump("cT", cT)

    # ---- output projection (natural orientation) ----
    woutp = w_tile("w_outp")
    if "woutp" in dbg_outs:
        dma(dbg_outs["woutp"].ap()[:], woutp[:])
    bob = tl.tile([128, H], F32, tag="bob")
    for n in range(2):
        nsl = slice(n * NC, (n + 1) * NC)
        bc_ps = sps.tile([128, NC], F32, tag="bb")
        nc.tensor.matmul(bc_ps[:], ones_bfrow[:], b_outp_row[:, nsl],
                         start=True, stop=True)
        nc.scalar.copy(bob[:, nsl], bc_ps[:])

    if "bob" in dbg_outs:
        dma(dbg_outs["bob"].ap()[:], bob[:])
    for st in range(ST):
        ssl = slice(st * 128, (st + 1) * 128)
        o_sb = obp.tile([128, H], F32, tag="o")
        for n in range(2):
            nsl = slice(n * NC, (n + 1) * NC)
            ps = ps2.tile([128, NC], F32, tag="g")
            for k in range(KT):
                nc.tensor.matmul(ps[:], cT[:, k, ssl], woutp[:, k, nsl],
                                 start=(k == 0), stop=(k == KT - 1))
            nc.vector.tensor_tensor(o_sb[:, nsl], ps[:], bob[:, nsl], op=ALU.add)
        dma(out_p.ap()[ssl, :], o_sb[:])

    # ---- uncertainty head ----
    wu1 = tl.tile([128, KT, 512], BF16, tag="wu1")
    dma(wu1[:], di["wu1"].ap()[:])
    wu2 = tl.tile([128, 4], BF16, tag="wu2")
    dma(wu2[:], di["wu2"].ap()[:])

    cm = tl.tile([128, KT], F32, tag="cm")
    for k in range(KT):
        nc.vector.reduce_sum(cm[:, k:k + 1], cT[:, k, :], axis=AX.X)
    nc.vector.tensor_scalar_mul(cm[:], cm[:], 1.0 / S)
    cmb = tl.tile([128, KT], BF16, tag="cmb")
    nc.vector.tensor_copy(cmb[:], cm[:])

    pm = tl.tile([128, KT], BF16, tag="pm")
    for m in range(KT):
        ps = sps.tile([128, 1], F32, tag="pmp")
        for k in range(KT):
            nc.tensor.matmul(ps[:], woutp[:, k, m * 128:(m + 1) * 128],
                             cmb[:, k:k + 1], start=(k == 0), stop=(k == KT - 1))
        nc.scalar.activation(pm[:, m:m + 1], ps[:], AF.Identity,
                             bias=b_outp[:, m:m + 1])
    u_sb = tl.tile([128, 4], BF16, tag="u")
    for mu in range(4):
        ps = sps.tile([128, 1], F32, tag="pmp")
        for k in range(KT):
            nc.tensor.matmul(ps[:], wu1[:, k, mu * 128:(mu + 1) * 128],
                             pm[:, k:k + 1], start=(k == 0), stop=(k == KT - 1))
        nc.scalar.activation(u_sb[:, mu:mu + 1], ps[:], AF.Gelu_apprx_tanh,
                             bias=bu1[:, mu:mu + 1])
    un_ps = sps.tile([1, 1], F32, tag="un")
    for ku in range(4):
        nc.tensor.matmul(un_ps[:], u_sb[:, ku:ku + 1], wu2[:, ku:ku + 1],
                         start=(ku == 0), stop=(ku == 3))
    un_sb = tl.tile([1, 1], F32, tag="uns")
    nc.scalar.activation(un_sb[:], un_ps[:], AF.Sigmoid, bias=bu2[:])
    dma(out_unc.ap()[:], un_sb[:])

    sps_cm.__exit__(None, None, None)
    ps2_cm.__exit__(None, None, None)
    ob_cm.__exit__(None, None, None)
    tl_cm.__exit__(None, None, None)
    ymp_cm.__exit__(None, None, None)
    wts_cm.__exit__(None, None, None)
    big_cm.__exit__(None, None, None)
    cpool_cm.__exit__(None, None, None)


def _prep_inputs(inputs):
    f = {k: np.asarray(v, np.float32) for k, v in inputs.items()}
    Wo = f["Wo"]
    bo_eff = f["bo"] + f["bv"] @ Wo

    shared = {
        "w_in": _t8(_bf(f["W_in"])),
        "wq": _t8(_bf(f["Wq"])),
        "wk": _t8(_bf(f["Wk"])),
        "wv": _t8(_bf(f["Wv"])),
        "wo": _t8(_bf(Wo)),
        "w_outp": _t8(_bf(f["W_outp"])),
        "w1": np.ascontiguousarray(
            _bf(f["W1"]).reshape(8, 128, DT, 128).transpose(1, 2, 0, 3)),
        "w2": np.ascontiguousarray(
            _bf(f["W2"]).reshape(DT, 128, H).transpose(1, 0, 2)),
        "w_heads": np.ascontiguousarray(
            _bf(f["W_heads"]).reshape(ND, 8, 128, H).transpose(0, 2, 1, 3)),
        "wu1": _t8(_bf(f["Wu1"])),
        "wu2": np.ascontiguousarray(_bf(f["Wu2"]).reshape(4, 128).T),
        "w_dom": _t8(f["W_dom"]),
        "cpack": np.concatenate(
            [_bvec(v, 8) for v in [f["b_in"], f["bq"], f["bk"], bo_eff, f["b2"],
                                   f["b_outp"], f["ln1_g"], f["ln1_b"],
                                   f["ln2_g"], f["ln2_b"]]]
            + [_bvec(f["b1"], DT), _bvec(f["bu1"], 4)], axis=1),
        "rpack": np.concatenate(
            [f["b_dom"].reshape(1, ND), np.arange(ND, dtype=np.float32).reshape(1, ND),
             f["bu2"].reshape(1, 1), np.zeros((1, 5), np.float32)], axis=1),
        "bh": np.ascontiguousarray(f["b_heads"].reshape(ND, 8, 128).transpose(0, 2, 1)),
        "b_outp_row": _bf(f["b_outp"]).reshape(1, H).copy(),
    }
    hs = f["hidden_states"]
    in_maps = []
    for b in range(B):
        m = dict(shared)
        m["hT"] = _t8(_bf(hs[b].T))
        in_maps.append(m)
    return in_maps


def _get_nc(dbg=False):
    key = ("nc", dbg)
    if key not in _CACHE:
        _CACHE[key] = build_nc(dbg)
    return _CACHE[key]


def run(inputs, trace=False, dbg=False, n_cores=B):
    from concourse.bass_utils import run_bass_kernel_spmd

    nc = _get_nc(dbg)
    in_maps = _prep_inputs(inputs)[:n_cores]
    res = run_bass_kernel_spmd(nc, in_maps, core_ids=list(range(n_cores)), trace=trace)
    processed = np.stack([r["out_p"] for r in res.results]).astype(np.float32)
    probs = np.stack([r["out_probs"][0] for r in res.results]).astype(np.float32)
    idx = np.array([r["out_idx"][0, 0] for r in res.results], np.int32)
    unc = np.stack([r["out_unc"][0] for r in res.results]).astype(np.float32)
    return (processed, probs, idx, unc), res


def kernel(**inputs):
    outs, _ = run(inputs, trace=False)
    return outs


if __name__ == "__main__":
    if os.environ.get("BUILD_ONLY"):
        build_nc()
        print("build ok")
